# revision 10
# baseline (speedup 1.0000x reference)
"""Trainium2 Bass kernel for nn_DendriticANN.

Network (reference.py):
    h = BN(leaky(x @ W_in.T + b_in))                       [B, H]
    for l in range(L):
        xn   = h / max(||h||_row, 1e-12)                   row-wise L2 normalize
        dend = leaky(einsum('bi,ndi->bnd', xn, Wd[l]))     [B, H, D]
        out  = leaky(einsum('bnd,nd->bn', dend, soma[l]))  [B, H]
        h    = BN(leaky(out))
    y = h @ W_out.T + b_out                                [B, OUT]

Sharding: data-parallel over batch across 8 cores (B=2048 -> 256 rows/core),
all parameters replicated.  Everything on-chip uses a [features, batch]
layout so BatchNorm reductions are free-axis native and layer outputs feed
the next layer's matmul K-tiles without any transposes.  BatchNorm batch
stats are combined with one 4 KB AllReduce per BN (3 total).

The dendritic einsum is a plain matmul [B,H] @ [H, H*D] with the weight
columns ordered d-major (nd = d*512 + n), so each 128-row tile of the
output holds one dendrite-index d for 128 neurons; the soma reduction over
d then becomes PSUM accumulation of diag(soma[:,d]) @ tile matmuls, with
the diagonal matrices built on device (identity * soma column on DVE).

Matmuls run in float32r (TF32-like, full PE speed, fp32 memory format).

Workaround: this walrus build rejects instructions carrying more than one
sync wait ("Too many sync wait commands"), but Tile's wait assignment
attaches one wait per producer semaphore.  Before every compile we rewrite
the BIR JSON, moving excess waits onto same-engine NoOps inserted right
before the owning instruction.
"""

import json

import numpy as np

import concourse.bass as bass
import concourse.mybir as mybir
import concourse.tile as tile
from concourse.bass_utils import run_bass_kernel_spmd

# ---------------------------------------------------------------- problem dims
N_CORES = 8
B, IN, H, D, OUT, L = 2048, 1024, 512, 32, 10, 2
BL = B // N_CORES            # 256 batch rows per core
ND = H * D                   # 16384 dendrite columns per layer
NG = H // 128                # 4 feature groups of 128
KD = H // 128                # 4 K-tiles for the dendritic matmul
BN_EPS = 1e-5
SLOPE = 0.01
F32 = mybir.dt.float32
F32R = mybir.dt.float32r

WCOLS = 2048                 # weight DMA chunk: [128, WCOLS] = 1 MB
NCHUNK = ND // WCOLS         # 8 column chunks per layer
TPC = WCOLS // 128           # 16 nd-tiles per chunk

# ------------------------------------------------- walrus 1-wait workaround


_patch_state = {"installed": False, "counter": 0}


def _split_excess_waits(bir_json):
    m = json.loads(bir_json)
    moved = 0
    for func in m.get("functions", []):
        for blk in func.get("blocks", []):
            new_insts = []
            for inst in blk.get("instructions", []):
                si = inst.get("sync_info") or {}
                waits = si.get("on_wait") or []
                if len(waits) > 1:
                    for w in waits[:-1]:
                        _patch_state["counter"] += 1
                        new_insts.append({
                            "opcode": "NoOp",
                            "name": f"I-waitsplit-{_patch_state['counter']}",
                            "engine": inst.get("engine", "SP"),
                            "ins": [],
                            "outs": [],
                            "debug": inst.get("debug", 0),
                            "sync_info": {"on_wait": [w], "on_update": []},
                        })
                        moved += 1
                    si["on_wait"] = [waits[-1]]
                    inst["sync_info"] = si
                new_insts.append(inst)
            blk["instructions"] = new_insts
    return json.dumps(m).encode(), moved


def _install_compile_patch():
    if _patch_state["installed"]:
        return
    _patch_state["installed"] = True
    import concourse.bass_utils as bass_utils
    import concourse.bass2jax as bass2jax

    orig = bass_utils.compile_bir_kernel

    def patched(bir_json, tmpdir, neff_name="file.neff"):
        if isinstance(bir_json, str):
            bir_json = bir_json.encode()
        bir_json, _ = _split_excess_waits(bir_json)
        return orig(bir_json, tmpdir, neff_name)

    bass_utils.compile_bir_kernel = patched
    bass2jax.compile_bir_kernel = patched


_install_compile_patch()

# ------------------------------------------------------------------ bass build


def _batch_norm_stats_to_affine(nc, vec, stats_sb, g, inv_b):
    """Per-partition BN affine from (sum, sumsq) columns: scale, bias [128,1]."""
    mean = vec.tile([128, 1], F32, tag="bn_mean")
    ex2 = vec.tile([128, 1], F32, tag="bn_ex2")
    nc.vector.tensor_scalar_mul(mean[:], stats_sb[:, 2 * g:2 * g + 1], inv_b)
    nc.vector.tensor_scalar_mul(ex2[:], stats_sb[:, 2 * g + 1:2 * g + 2], inv_b)
    var = vec.tile([128, 1], F32, tag="bn_var")
    msq = vec.tile([128, 1], F32, tag="bn_msq")
    nc.vector.tensor_tensor(msq[:], mean[:], mean[:], mybir.AluOpType.mult)
    nc.vector.tensor_tensor(var[:], ex2[:], msq[:], mybir.AluOpType.subtract)
    vare = vec.tile([128, 1], F32, tag="bn_vare")
    nc.vector.tensor_scalar_add(vare[:], var[:], BN_EPS)
    denom = vec.tile([128, 1], F32, tag="bn_denom")
    nc.scalar.activation(denom[:], vare[:], mybir.ActivationFunctionType.Sqrt)
    scale = vec.tile([128, 1], F32, tag="bn_scale")
    nc.vector.reciprocal(scale[:], denom[:])
    negm = vec.tile([128, 1], F32, tag="bn_negm")
    nc.vector.tensor_scalar_mul(negm[:], mean[:], -1.0)
    bias = vec.tile([128, 1], F32, tag="bn_bias")
    nc.vector.tensor_tensor(bias[:], negm[:], scale[:], mybir.AluOpType.mult)
    return scale, bias


def build_nc(mm_dt=F32R):
    nc = bass.Bass(num_devices=N_CORES)

    xT = nc.dram_tensor("xT", [IN, BL], mm_dt, kind="ExternalInput")
    w_inT = nc.dram_tensor("w_inT", [IN, H], mm_dt, kind="ExternalInput")
    b_in = nc.dram_tensor("b_in", [H, 1], F32, kind="ExternalInput")
    wd = nc.dram_tensor("wd", [L, H, ND], mm_dt, kind="ExternalInput")
    soma_d = nc.dram_tensor("soma", [L, NG, 128, D], F32, kind="ExternalInput")
    w_outT = nc.dram_tensor("w_outT", [H, OUT], mm_dt, kind="ExternalInput")
    b_out = nc.dram_tensor("b_out", [OUT, 1], F32, kind="ExternalInput")
    ident_d = nc.dram_tensor("ident", [128, 128], F32, kind="ExternalInput")
    ones_col_d = nc.dram_tensor("ones_col", [128, 1], mm_dt, kind="ExternalInput")
    ones_row_d = nc.dram_tensor("ones_row", [1, 128], mm_dt, kind="ExternalInput")
    y = nc.dram_tensor("y", [OUT, BL], F32, kind="ExternalOutput")

    inv_b = 1.0 / B
    Lrelu = mybir.ActivationFunctionType.Lrelu
    Prelu = mybir.ActivationFunctionType.Prelu
    Ident = mybir.ActivationFunctionType.Identity
    Square = mybir.ActivationFunctionType.Square
    Sqrt = mybir.ActivationFunctionType.Sqrt

    with tile.TileContext(nc) as tc:
        with (
            tc.tile_pool(name="const", bufs=1) as constp,
            tc.tile_pool(name="wstream", bufs=2 * KD) as wstream,
            tc.tile_pool(name="acts", bufs=2) as acts,            # lq/h/xn per group
            tc.tile_pool(name="work", bufs=4) as work,            # ld, diag, junk
            tc.tile_pool(name="vec", bufs=4) as vec,             # [128,1]-ish stats
            tc.tile_pool(name="psum_d", bufs=3, space="PSUM") as psum_d_p,
            tc.tile_pool(name="psum_q", bufs=NG, space="PSUM") as psum_q_p,
            tc.tile_pool(name="dram", bufs=2 * 3, space="DRAM") as dramp,
        ):
            # ---------------- constants
            ident_sb = constp.tile([128, 128], F32)
            nc.sync.dma_start(ident_sb[:], ident_d[:])
            ones_col = constp.tile([128, 1], mm_dt)
            nc.sync.dma_start(ones_col[:], ones_col_d[:])
            ones_row = constp.tile([1, 128], mm_dt)
            nc.sync.dma_start(ones_row[:], ones_row_d[:])
            b_in_tiles = []
            for g in range(NG):
                t = constp.tile([128, 1], F32, tag=f"b_in_{g}")
                nc.sync.dma_start(t[:], b_in[128 * g:128 * (g + 1), :])
                b_in_tiles.append(t)
            b_out_sb = constp.tile([OUT, 1], F32)
            nc.sync.dma_start(b_out_sb[:], b_out[:])
            w_out_tiles = []
            for g in range(NG):
                t = constp.tile([128, OUT], mm_dt, tag=f"w_out_{g}")
                nc.sync.dma_start(t[:], w_outT[128 * g:128 * (g + 1), :])
                w_out_tiles.append(t)
            soma_tiles = {}
            for l in range(L):
                for g in range(NG):
                    t = constp.tile([128, D], F32, tag=f"soma_{l}_{g}")
                    nc.sync.dma_start(t[:], soma_d[l, g])
                    soma_tiles[(l, g)] = t

            w_in_tiles = []
            for k in range(IN // 128):
                t = constp.tile([128, H], mm_dt, tag=f"w_in_{k}")
                nc.sync.dma_start(t[:], w_inT[128 * k:128 * (k + 1), :])
                w_in_tiles.append(t)
            xT_tiles = []
            for k in range(IN // 128):
                t = constp.tile([128, BL], mm_dt, tag=f"xT_{k}")
                nc.sync.dma_start(t[:], xT[128 * k:128 * (k + 1), :])
                xT_tiles.append(t)

            def bn_block(lq_tiles, need_xn):
                """Shared BN + (optional) L2-normalize tail.

                lq_tiles: NG tiles [128, BL] holding leaky(pre-BN) activations,
                each already carrying its accum_out sum in stats_sb col 2g.
                Returns (h_tiles, xn_tiles or None).
                """
                pass  # replaced below; kept for readability

            # ---------------- per-BN-stage pipeline (stage 0 + L layers)
            xn_tiles = None   # rhs K-tiles for next matmul
            h_tiles = None

            for stage in range(L + 1):
                stats_sb = vec.tile([128, 2 * NG], F32, tag="stats")
                lq_tiles = []

                if stage == 0:
                    # input layer: psum[g] = sum_k w_inT[k,g].T @ xT[k]
                    for g in range(NG):
                        ps = psum_q_p.tile([128, BL], F32, tag="psum_q")
                        for k in range(IN // 128):
                            nc.tensor.matmul(
                                ps[:], w_in_tiles[k][:, 128 * g:128 * (g + 1)],
                                xT_tiles[k][:],
                                start=(k == 0), stop=(k == IN // 128 - 1))
                        lq = acts.tile([128, BL], mm_dt, tag=f"lq{g}")
                        nc.scalar.activation(
                            lq[:], ps[:], Lrelu,
                            bias=b_in_tiles[g][:], alpha=SLOPE,
                            accum_out=stats_sb[:, 2 * g:2 * g + 1])
                        lq_tiles.append(lq)
                else:
                    l = stage - 1
                    # dendritic matmul + diag-soma PSUM reduction
                    psq = [psum_q_p.tile([128, BL], F32, tag="psum_q", name=f"psq{_g}")
                           for _g in range(NG)]
                    for cc in range(NCHUNK):
                        wk = []
                        for k in range(KD):
                            w = wstream.tile([128, WCOLS], mm_dt, tag="wchunk")
                            nc.sync.dma_start(
                                w[:], wd[l, 128 * k:128 * (k + 1),
                                         WCOLS * cc:WCOLS * (cc + 1)])
                            wk.append(w)
                        for tp in range(TPC // 2):
                            ps = psum_d_p.tile([128, 2 * BL], F32, tag="psum_d")
                            for half in range(2):
                                tt = 2 * tp + half
                                for k in range(KD):
                                    nc.tensor.matmul(
                                        ps[:, BL * half:BL * (half + 1)],
                                        wk[k][:, 128 * tt:128 * (tt + 1)],
                                        xn_tiles[k][:],
                                        start=(k == 0), stop=(k == KD - 1))
                            ld = work.tile([128, 2 * BL], mm_dt, tag="ld")
                            nc.scalar.activation(ld[:], ps[:], Lrelu, alpha=SLOPE)
                            for half in range(2):
                                t_glob = cc * TPC + 2 * tp + half
                                d_idx, nb = divmod(t_glob, NG)
                                diag = work.tile([128, 128], mm_dt, tag="diag")
                                nc.vector.tensor_scalar_mul(
                                    diag[:], ident_sb[:],
                                    soma_tiles[(l, nb)][:, d_idx:d_idx + 1])
                                nc.tensor.matmul(
                                    psq[nb][:], diag[:],
                                    ld[:, BL * half:BL * (half + 1)],
                                    start=(d_idx == 0), stop=(d_idx == D - 1),
                                    skip_group_check=True)
                    for g in range(NG):
                        lq = acts.tile([128, BL], mm_dt, tag=f"lq{g}")
                        # reference applies leaky twice here (soma output then
                        # again before BN): leaky∘leaky == Lrelu(slope^2)
                        nc.scalar.activation(
                            lq[:], psq[g][:], Prelu, alpha=SLOPE * SLOPE,
                            accum_out=stats_sb[:, 2 * g:2 * g + 1])
                        lq_tiles.append(lq)

                # ---- sumsq for BN var
                for g in range(NG):
                    junk = work.tile([128, BL], F32, tag="junk")
                    nc.scalar.activation(
                        junk[:], lq_tiles[g][:], Square,
                        accum_out=stats_sb[:, 2 * g + 1:2 * g + 2])

                # ---- AllReduce batch stats across cores
                st_in = dramp.tile([128, 2 * NG], F32, tag="st_in")
                st_out = dramp.tile([N_CORES, 128, 2 * NG], F32, tag="st_out")
                nc.sync.dma_start(st_in[:], stats_sb[:])
                nc.gpsimd.collective_compute(
                    "AllGather", mybir.AluOpType.bypass,
                    replica_groups=[list(range(N_CORES))],
                    ins=[st_in.opt()], outs=[st_out.opt()],
                )
                stats_all = vec.tile([128, N_CORES * 2 * NG], F32,
                                     tag="stats_all")
                nc.sync.dma_start(
                    stats_all[:].rearrange("p (r c) -> p r c", r=N_CORES),
                    st_out[:].rearrange("r p c -> p r c"))
                stats_g = vec.tile([128, 2 * NG], F32, tag="stats_g")
                nc.vector.tensor_reduce(
                    stats_g[:],
                    stats_all[:].rearrange("p (r c) -> p c r", r=N_CORES),
                    mybir.AxisListType.X, mybir.AluOpType.add)

                # ---- BN apply (+ hsq for L2 when another layer follows)
                need_xn = stage < L
                h_tiles = []
                hsq_tiles = []
                for g in range(NG):
                    scale, bias = _batch_norm_stats_to_affine(
                        nc, vec, stats_g, g, inv_b)
                    h = acts.tile([128, BL], mm_dt, tag=f"h{g}")
                    nc.scalar.activation(h[:], lq_tiles[g][:], Ident,
                                         bias=bias[:], scale=scale[:])
                    h_tiles.append(h)
                    if need_xn:
                        hsq = work.tile([128, BL], mm_dt, tag="junk")
                        nc.scalar.activation(hsq[:], lq_tiles[g][:], Square,
                                             bias=bias[:], scale=scale[:])
                        hsq_tiles.append(hsq)

                if need_xn:
                    # ---- row L2 norm: rinv[b] = 1/sqrt(max(sum_f h^2, eps))
                    ps_r = psum_d_p.tile([1, BL], F32, tag="psum_d")
                    for g in range(NG):
                        nc.tensor.matmul(ps_r[:], ones_col[:], hsq_tiles[g][:],
                                         start=(g == 0), stop=(g == NG - 1))
                    ssq = vec.tile([1, BL], F32, tag="ssq")
                    nc.vector.tensor_scalar_max(ssq[:], ps_r[:], 1e-24)
                    rnorm = vec.tile([1, BL], F32, tag="rnorm")
                    nc.scalar.activation(rnorm[:], ssq[:], Sqrt)
                    rinv = vec.tile([1, BL], mm_dt, tag="rinv")
                    with nc.allow_low_precision(
                            reason="rinv feeds fp32r matmul; fp32r rounding ok"):
                        nc.vector.reciprocal(rinv[:], rnorm[:])
                    # broadcast rinv across partitions via K=1 outer product
                    ps_b = psum_d_p.tile([128, BL], F32, tag="psum_d")
                    nc.tensor.matmul(ps_b[:], ones_row[:], rinv[:],
                                     start=True, stop=True)
                    xn_tiles = []
                    for g in range(NG):
                        xn = acts.tile([128, BL], mm_dt, tag=f"xn{g}")
                        nc.vector.tensor_tensor(xn[:], h_tiles[g][:], ps_b[:],
                                                mybir.AluOpType.mult)
                        xn_tiles.append(xn)

            # ---------------- output layer: y = h @ W_out.T + b_out
            ps_y = psum_d_p.tile([OUT, BL], F32, tag="psum_d")
            for g in range(NG):
                nc.tensor.matmul(ps_y[:], w_out_tiles[g][:],
                                 h_tiles[g][:], start=(g == 0), stop=(g == NG - 1))
            y_sb = work.tile([OUT, BL], F32, tag="ld")
            nc.scalar.activation(y_sb[:], ps_y[:], Ident, bias=b_out_sb[:])
            nc.sync.dma_start(y[:], y_sb[:])

    return nc


# ------------------------------------------------------------------ host side

_cache = {}


def _get_nc():
    if "nc" not in _cache:
        _cache["nc"] = build_nc()
    return _cache["nc"]


def make_in_maps(x, W_in, b_in, Wd, soma, W_out, b_out):
    xT = np.ascontiguousarray(x.T, dtype=np.float32)
    w_inT = np.ascontiguousarray(W_in.T, dtype=np.float32)
    wd2 = np.ascontiguousarray(
        Wd.transpose(0, 3, 2, 1).reshape(L, H, ND), dtype=np.float32)
    soma2 = np.ascontiguousarray(
        soma.reshape(L, NG, 128, D), dtype=np.float32)
    w_outT = np.ascontiguousarray(W_out.T, dtype=np.float32)
    common = dict(
        w_inT=w_inT,
        b_in=np.ascontiguousarray(b_in.reshape(H, 1), dtype=np.float32),
        wd=wd2,
        soma=soma2,
        w_outT=w_outT,
        b_out=np.ascontiguousarray(b_out.reshape(OUT, 1), dtype=np.float32),
        ident=np.eye(128, dtype=np.float32),
        ones_col=np.ones((128, 1), dtype=np.float32),
        ones_row=np.ones((1, 128), dtype=np.float32),
    )
    in_maps = []
    for c in range(N_CORES):
        m = dict(common)
        m["xT"] = np.ascontiguousarray(xT[:, BL * c:BL * (c + 1)])
        in_maps.append(m)
    return in_maps


def kernel(x, W_in, b_in, Wd, soma, W_out, b_out):
    x = np.asarray(x)
    in_maps = make_in_maps(np.asarray(x, dtype=np.float32),
                           np.asarray(W_in), np.asarray(b_in),
                           np.asarray(Wd), np.asarray(soma),
                           np.asarray(W_out), np.asarray(b_out))
    nc = _get_nc()
    res = run_bass_kernel_spmd(nc, in_maps, core_ids=list(range(N_CORES)))
    y = np.concatenate([r["y"] for r in res.results], axis=1)  # [OUT, B]
    return np.ascontiguousarray(y.T, dtype=np.float32)


if __name__ == "__main__":
    rng = np.random.default_rng(0)
    x = rng.standard_normal((B, IN), dtype=np.float32)
    W_in = (rng.standard_normal((H, IN), dtype=np.float32) / np.sqrt(IN))
    b_in_a = np.zeros(H, np.float32)
    Wd_a = rng.standard_normal((L, H, D, H), dtype=np.float32) * 0.1
    soma_a = rng.standard_normal((L, H, D), dtype=np.float32) * 0.1
    W_out = rng.standard_normal((OUT, H), dtype=np.float32) / np.sqrt(H)
    b_out_a = np.zeros(OUT, np.float32)
    y = kernel(x=x, W_in=W_in, b_in=b_in_a, Wd=Wd_a, soma=soma_a,
               W_out=W_out, b_out=b_out_a)
    print("kernel output:", y.shape, y.dtype, float(np.abs(y).max()))


# revision 12
# speedup vs baseline: 2.0226x; 2.0226x over previous
"""Trainium2 Bass kernel for nn_DendriticANN.

Network (reference.py):
    h = BN(leaky(x @ W_in.T + b_in))                       [B, H]
    for l in range(L):
        xn   = h / max(||h||_row, 1e-12)                   row-wise L2 normalize
        dend = leaky(einsum('bi,ndi->bnd', xn, Wd[l]))     [B, H, D]
        out  = leaky(einsum('bnd,nd->bn', dend, soma[l]))  [B, H]
        h    = BN(leaky(out))
    y = h @ W_out.T + b_out                                [B, OUT]

Sharding: data-parallel over batch across 8 cores (B=2048 -> 256 rows/core),
all parameters replicated.  Everything on-chip uses a [features, batch]
layout so BatchNorm reductions are free-axis native and layer outputs feed
the next layer's matmul K-tiles without any transposes.  BatchNorm batch
stats are combined with one 4 KB AllReduce per BN (3 total).

The dendritic einsum is a plain matmul [B,H] @ [H, H*D] with the weight
columns ordered d-major (nd = d*512 + n), so each 128-row tile of the
output holds one dendrite-index d for 128 neurons; the soma reduction over
d then becomes PSUM accumulation of diag(soma[:,d]) @ tile matmuls, with
the diagonal matrices built on device (identity * soma column on DVE).

Matmuls run in float32r (TF32-like, full PE speed, fp32 memory format).

Workaround: this walrus build rejects instructions carrying more than one
sync wait ("Too many sync wait commands"), but Tile's wait assignment
attaches one wait per producer semaphore.  Before every compile we rewrite
the BIR JSON, moving excess waits onto same-engine NoOps inserted right
before the owning instruction.
"""

import json

import numpy as np

import concourse.bass as bass
import concourse.mybir as mybir
import concourse.tile as tile
from concourse.bass_utils import run_bass_kernel_spmd

# ---------------------------------------------------------------- problem dims
N_CORES = 8
B, IN, H, D, OUT, L = 2048, 1024, 512, 32, 10, 2
BL = B // N_CORES            # 256 batch rows per core
ND = H * D                   # 16384 dendrite columns per layer
NG = H // 128                # 4 feature groups of 128
KD = H // 128                # 4 K-tiles for the dendritic matmul
BN_EPS = 1e-5
SLOPE = 0.01
F32 = mybir.dt.float32
F32R = mybir.dt.float32r
BF16 = mybir.dt.bfloat16
import os as _os
MM_DT = BF16 if _os.environ.get("KERNEL_MM_DT", "f32r") == "bf16" else F32R

WCOLS = 2048                 # weight DMA chunk: [128, WCOLS] = 1 MB
NCHUNK = ND // WCOLS         # 8 column chunks per layer
TPC = WCOLS // 128           # 16 nd-tiles per chunk

# ------------------------------------------------- walrus 1-wait workaround


_patch_state = {"installed": False, "counter": 0}


def _split_excess_waits(bir_json):
    m = json.loads(bir_json)
    moved = 0
    for func in m.get("functions", []):
        for blk in func.get("blocks", []):
            new_insts = []
            for inst in blk.get("instructions", []):
                si = inst.get("sync_info") or {}
                waits = si.get("on_wait") or []
                if len(waits) > 1:
                    for w in waits[:-1]:
                        _patch_state["counter"] += 1
                        new_insts.append({
                            "opcode": "NoOp",
                            "name": f"I-waitsplit-{_patch_state['counter']}",
                            "engine": inst.get("engine", "SP"),
                            "ins": [],
                            "outs": [],
                            "debug": inst.get("debug", 0),
                            "sync_info": {"on_wait": [w], "on_update": []},
                        })
                        moved += 1
                    si["on_wait"] = [waits[-1]]
                    inst["sync_info"] = si
                new_insts.append(inst)
            blk["instructions"] = new_insts
    return json.dumps(m).encode(), moved


def _install_compile_patch():
    if _patch_state["installed"]:
        return
    _patch_state["installed"] = True
    import concourse.bass_utils as bass_utils
    import concourse.bass2jax as bass2jax

    orig = bass_utils.compile_bir_kernel

    def patched(bir_json, tmpdir, neff_name="file.neff"):
        if isinstance(bir_json, str):
            bir_json = bir_json.encode()
        bir_json, _ = _split_excess_waits(bir_json)
        return orig(bir_json, tmpdir, neff_name)

    bass_utils.compile_bir_kernel = patched
    bass2jax.compile_bir_kernel = patched


_install_compile_patch()

# ------------------------------------------------------------------ bass build


def _batch_norm_stats_to_affine(nc, vec, stats_sb, g, inv_b):
    """Per-partition BN affine from (sum, sumsq) columns: scale, bias [128,1]."""
    mean = vec.tile([128, 1], F32, tag="bn_mean")
    ex2 = vec.tile([128, 1], F32, tag="bn_ex2")
    nc.vector.tensor_scalar_mul(mean[:], stats_sb[:, 2 * g:2 * g + 1], inv_b)
    nc.vector.tensor_scalar_mul(ex2[:], stats_sb[:, 2 * g + 1:2 * g + 2], inv_b)
    var = vec.tile([128, 1], F32, tag="bn_var")
    msq = vec.tile([128, 1], F32, tag="bn_msq")
    nc.vector.tensor_tensor(msq[:], mean[:], mean[:], mybir.AluOpType.mult)
    nc.vector.tensor_tensor(var[:], ex2[:], msq[:], mybir.AluOpType.subtract)
    vare = vec.tile([128, 1], F32, tag="bn_vare")
    nc.vector.tensor_scalar_add(vare[:], var[:], BN_EPS)
    denom = vec.tile([128, 1], F32, tag="bn_denom")
    nc.scalar.activation(denom[:], vare[:], mybir.ActivationFunctionType.Sqrt)
    scale = vec.tile([128, 1], F32, tag="bn_scale")
    nc.vector.reciprocal(scale[:], denom[:])
    negm = vec.tile([128, 1], F32, tag="bn_negm")
    nc.vector.tensor_scalar_mul(negm[:], mean[:], -1.0)
    bias = vec.tile([128, 1], F32, tag="bn_bias")
    nc.vector.tensor_tensor(bias[:], negm[:], scale[:], mybir.AluOpType.mult)
    return scale, bias


def build_nc(mm_dt=None):
    if mm_dt is None:
        mm_dt = MM_DT
    nc = bass.Bass(num_devices=N_CORES)

    xT = nc.dram_tensor("xT", [IN, BL], mm_dt, kind="ExternalInput")
    w_inT = nc.dram_tensor("w_inT", [IN, H], mm_dt, kind="ExternalInput")
    b_in = nc.dram_tensor("b_in", [H, 1], F32, kind="ExternalInput")
    wd = nc.dram_tensor("wd", [L, H, ND], mm_dt, kind="ExternalInput")
    soma_d = nc.dram_tensor("soma", [L, NG, 128, D], F32, kind="ExternalInput")
    w_outT = nc.dram_tensor("w_outT", [H, OUT], mm_dt, kind="ExternalInput")
    b_out = nc.dram_tensor("b_out", [OUT, 1], F32, kind="ExternalInput")
    ident_d = nc.dram_tensor("ident", [128, 128], F32, kind="ExternalInput")
    ones_col_d = nc.dram_tensor("ones_col", [128, 1], mm_dt, kind="ExternalInput")
    ones_row_d = nc.dram_tensor("ones_row", [1, 128], mm_dt, kind="ExternalInput")
    y = nc.dram_tensor("y", [OUT, BL], F32, kind="ExternalOutput")

    inv_b = 1.0 / B
    Lrelu = mybir.ActivationFunctionType.Lrelu
    Prelu = mybir.ActivationFunctionType.Prelu
    Ident = mybir.ActivationFunctionType.Identity
    Square = mybir.ActivationFunctionType.Square
    Sqrt = mybir.ActivationFunctionType.Sqrt

    with tile.TileContext(nc) as tc:
        with (
            tc.tile_pool(name="const", bufs=1) as constp,
            tc.tile_pool(name="wstream", bufs=2 * KD) as wstream,
            tc.tile_pool(name="acts", bufs=2) as acts,            # lq/h/xn per group
            tc.tile_pool(name="work", bufs=4) as work,            # ld, diag, junk
            tc.tile_pool(name="vec", bufs=4) as vec,             # [128,1]-ish stats
            tc.tile_pool(name="psum_d", bufs=3, space="PSUM") as psum_d_p,
            tc.tile_pool(name="psum_q", bufs=NG, space="PSUM") as psum_q_p,
            tc.tile_pool(name="dram", bufs=2 * 3, space="DRAM") as dramp,
        ):
            # ---------------- constants
            ident_sb = constp.tile([128, 128], F32)
            nc.sync.dma_start(ident_sb[:], ident_d[:])
            ones_col = constp.tile([128, 1], mm_dt)
            nc.sync.dma_start(ones_col[:], ones_col_d[:])
            ones_row = constp.tile([1, 128], mm_dt)
            nc.sync.dma_start(ones_row[:], ones_row_d[:])
            b_in_tiles = []
            for g in range(NG):
                t = constp.tile([128, 1], F32, tag=f"b_in_{g}")
                nc.sync.dma_start(t[:], b_in[128 * g:128 * (g + 1), :])
                b_in_tiles.append(t)
            b_out_sb = constp.tile([OUT, 1], F32)
            nc.sync.dma_start(b_out_sb[:], b_out[:])
            w_out_tiles = []
            for g in range(NG):
                t = constp.tile([128, OUT], mm_dt, tag=f"w_out_{g}")
                nc.sync.dma_start(t[:], w_outT[128 * g:128 * (g + 1), :])
                w_out_tiles.append(t)
            soma_tiles = {}
            for l in range(L):
                for g in range(NG):
                    t = constp.tile([128, D], F32, tag=f"soma_{l}_{g}")
                    nc.sync.dma_start(t[:], soma_d[l, g])
                    soma_tiles[(l, g)] = t

            w_in_tiles = []
            for k in range(IN // 128):
                t = constp.tile([128, H], mm_dt, tag=f"w_in_{k}")
                nc.sync.dma_start(t[:], w_inT[128 * k:128 * (k + 1), :])
                w_in_tiles.append(t)
            xT_tiles = []
            for k in range(IN // 128):
                t = constp.tile([128, BL], mm_dt, tag=f"xT_{k}")
                nc.sync.dma_start(t[:], xT[128 * k:128 * (k + 1), :])
                xT_tiles.append(t)

            def bn_block(lq_tiles, need_xn):
                """Shared BN + (optional) L2-normalize tail.

                lq_tiles: NG tiles [128, BL] holding leaky(pre-BN) activations,
                each already carrying its accum_out sum in stats_sb col 2g.
                Returns (h_tiles, xn_tiles or None).
                """
                pass  # replaced below; kept for readability

            # ---------------- per-BN-stage pipeline (stage 0 + L layers)
            xn_tiles = None   # rhs K-tiles for next matmul
            h_tiles = None

            for stage in range(L + 1):
                stats_sb = vec.tile([128, 2 * NG], F32, tag="stats")
                lq_tiles = []

                if stage == 0:
                    # input layer: psum[g] = sum_k w_inT[k,g].T @ xT[k]
                    for g in range(NG):
                        ps = psum_q_p.tile([128, BL], F32, tag="psum_q")
                        for k in range(IN // 128):
                            nc.tensor.matmul(
                                ps[:], w_in_tiles[k][:, 128 * g:128 * (g + 1)],
                                xT_tiles[k][:],
                                start=(k == 0), stop=(k == IN // 128 - 1))
                        lq = acts.tile([128, BL], mm_dt, tag=f"lq{g}")
                        nc.scalar.activation(
                            lq[:], ps[:], Lrelu,
                            bias=b_in_tiles[g][:], alpha=SLOPE,
                            accum_out=stats_sb[:, 2 * g:2 * g + 1])
                        lq_tiles.append(lq)
                else:
                    l = stage - 1
                    # dendritic matmul + diag-soma PSUM reduction
                    psq = [psum_q_p.tile([128, BL], F32, tag="psum_q", name=f"psq{_g}")
                           for _g in range(NG)]
                    def emit_q(pending):
                        ld_p, pair0 = pending
                        for half in range(2):
                            t_glob = pair0 + half
                            d_idx, nb = divmod(t_glob, NG)
                            diag = work.tile([128, 128], mm_dt, tag="diag")
                            nc.vector.tensor_scalar_mul(
                                diag[:], ident_sb[:],
                                soma_tiles[(l, nb)][:, d_idx:d_idx + 1])
                            nc.tensor.matmul(
                                psq[nb][:], diag[:],
                                ld_p[:, BL * half:BL * (half + 1)],
                                start=(d_idx == 0), stop=(d_idx == D - 1),
                                skip_group_check=True)

                    pending = None
                    for cc in range(NCHUNK):
                        wk = []
                        for k in range(KD):
                            w = wstream.tile([128, WCOLS], mm_dt, tag="wchunk")
                            nc.sync.dma_start(
                                w[:], wd[l, 128 * k:128 * (k + 1),
                                         WCOLS * cc:WCOLS * (cc + 1)])
                            wk.append(w)
                        for tp in range(TPC // 2):
                            ps = psum_d_p.tile([128, 2 * BL], F32, tag="psum_d")
                            for half in range(2):
                                tt = 2 * tp + half
                                for k in range(KD):
                                    nc.tensor.matmul(
                                        ps[:, BL * half:BL * (half + 1)],
                                        wk[k][:, 128 * tt:128 * (tt + 1)],
                                        xn_tiles[k][:],
                                        start=(k == 0), stop=(k == KD - 1))
                            ld = work.tile([128, 2 * BL], mm_dt, tag="ld")
                            nc.scalar.activation(ld[:], ps[:], Lrelu, alpha=SLOPE)
                            if pending is not None:
                                emit_q(pending)
                            pending = (ld, cc * TPC + 2 * tp)
                    emit_q(pending)
                    for g in range(NG):
                        lq = acts.tile([128, BL], mm_dt, tag=f"lq{g}")
                        # reference applies leaky twice here (soma output then
                        # again before BN): leaky∘leaky == Lrelu(slope^2)
                        nc.scalar.activation(
                            lq[:], psq[g][:], Prelu, alpha=SLOPE * SLOPE,
                            accum_out=stats_sb[:, 2 * g:2 * g + 1])
                        lq_tiles.append(lq)

                # ---- sumsq for BN var
                for g in range(NG):
                    junk = work.tile([128, BL], F32, tag="junk")
                    nc.scalar.activation(
                        junk[:], lq_tiles[g][:], Square,
                        accum_out=stats_sb[:, 2 * g + 1:2 * g + 2])

                # ---- AllReduce batch stats across cores
                st_in = dramp.tile([128, 2 * NG], F32, tag="st_in")
                st_out = dramp.tile([N_CORES, 128, 2 * NG], F32, tag="st_out")
                nc.sync.dma_start(st_in[:], stats_sb[:])
                nc.gpsimd.collective_compute(
                    "AllGather", mybir.AluOpType.bypass,
                    replica_groups=[list(range(N_CORES))],
                    ins=[st_in.opt()], outs=[st_out.opt()],
                )
                stats_all = vec.tile([128, N_CORES * 2 * NG], F32,
                                     tag="stats_all")
                nc.sync.dma_start(
                    stats_all[:].rearrange("p (r c) -> p r c", r=N_CORES),
                    st_out[:].rearrange("r p c -> p r c"))
                stats_g = vec.tile([128, 2 * NG], F32, tag="stats_g")
                nc.vector.tensor_reduce(
                    stats_g[:],
                    stats_all[:].rearrange("p (r c) -> p c r", r=N_CORES),
                    mybir.AxisListType.X, mybir.AluOpType.add)

                # ---- BN apply (+ hsq for L2 when another layer follows)
                need_xn = stage < L
                h_tiles = []
                hsq_tiles = []
                for g in range(NG):
                    scale, bias = _batch_norm_stats_to_affine(
                        nc, vec, stats_g, g, inv_b)
                    h = acts.tile([128, BL], mm_dt, tag=f"h{g}")
                    nc.scalar.activation(h[:], lq_tiles[g][:], Ident,
                                         bias=bias[:], scale=scale[:])
                    h_tiles.append(h)
                    if need_xn:
                        hsq = work.tile([128, BL], mm_dt, tag="junk")
                        nc.scalar.activation(hsq[:], lq_tiles[g][:], Square,
                                             bias=bias[:], scale=scale[:])
                        hsq_tiles.append(hsq)

                if need_xn:
                    # ---- row L2 norm: rinv[b] = 1/sqrt(max(sum_f h^2, eps))
                    ps_r = psum_d_p.tile([1, BL], F32, tag="psum_d")
                    for g in range(NG):
                        nc.tensor.matmul(ps_r[:], ones_col[:], hsq_tiles[g][:],
                                         start=(g == 0), stop=(g == NG - 1))
                    ssq = vec.tile([1, BL], F32, tag="ssq")
                    nc.vector.tensor_scalar_max(ssq[:], ps_r[:], 1e-24)
                    rnorm = vec.tile([1, BL], F32, tag="rnorm")
                    nc.scalar.activation(rnorm[:], ssq[:], Sqrt)
                    rinv = vec.tile([1, BL], mm_dt, tag="rinv")
                    with nc.allow_low_precision(
                            reason="rinv feeds fp32r matmul; fp32r rounding ok"):
                        nc.vector.reciprocal(rinv[:], rnorm[:])
                    # broadcast rinv across partitions via K=1 outer product
                    ps_b = psum_d_p.tile([128, BL], F32, tag="psum_d")
                    nc.tensor.matmul(ps_b[:], ones_row[:], rinv[:],
                                     start=True, stop=True)
                    xn_tiles = []
                    for g in range(NG):
                        xn = acts.tile([128, BL], mm_dt, tag=f"xn{g}")
                        nc.vector.tensor_tensor(xn[:], h_tiles[g][:], ps_b[:],
                                                mybir.AluOpType.mult)
                        xn_tiles.append(xn)

            # ---------------- output layer: y = h @ W_out.T + b_out
            ps_y = psum_d_p.tile([OUT, BL], F32, tag="psum_d")
            for g in range(NG):
                nc.tensor.matmul(ps_y[:], w_out_tiles[g][:],
                                 h_tiles[g][:], start=(g == 0), stop=(g == NG - 1))
            y_sb = work.tile([OUT, BL], F32, tag="ld")
            nc.scalar.activation(y_sb[:], ps_y[:], Ident, bias=b_out_sb[:])
            nc.sync.dma_start(y[:], y_sb[:])

    return nc


# ------------------------------------------------------------------ host side

_cache = {}


def _get_nc():
    if "nc" not in _cache:
        _cache["nc"] = build_nc()
    return _cache["nc"]


def make_in_maps(x, W_in, b_in, Wd, soma, W_out, b_out):
    mm_np = mybir.dt.np(MM_DT)
    xT = np.ascontiguousarray(x.T.astype(mm_np))
    w_inT = np.ascontiguousarray(W_in.T.astype(mm_np))
    wd2 = np.ascontiguousarray(
        Wd.transpose(0, 3, 2, 1).reshape(L, H, ND).astype(mm_np))
    soma2 = np.ascontiguousarray(
        soma.reshape(L, NG, 128, D), dtype=np.float32)
    w_outT = np.ascontiguousarray(W_out.T.astype(mm_np))
    common = dict(
        w_inT=w_inT,
        b_in=np.ascontiguousarray(b_in.reshape(H, 1), dtype=np.float32),
        wd=wd2,
        soma=soma2,
        w_outT=w_outT,
        b_out=np.ascontiguousarray(b_out.reshape(OUT, 1), dtype=np.float32),
        ident=np.eye(128, dtype=np.float32),
        ones_col=np.ones((128, 1), dtype=mm_np),
        ones_row=np.ones((1, 128), dtype=mm_np),
    )
    in_maps = []
    for c in range(N_CORES):
        m = dict(common)
        m["xT"] = np.ascontiguousarray(xT[:, BL * c:BL * (c + 1)])
        in_maps.append(m)
    return in_maps


def kernel(x, W_in, b_in, Wd, soma, W_out, b_out):
    x = np.asarray(x)
    in_maps = make_in_maps(np.asarray(x, dtype=np.float32),
                           np.asarray(W_in), np.asarray(b_in),
                           np.asarray(Wd), np.asarray(soma),
                           np.asarray(W_out), np.asarray(b_out))
    nc = _get_nc()
    res = run_bass_kernel_spmd(nc, in_maps, core_ids=list(range(N_CORES)))
    y = np.concatenate([r["y"] for r in res.results], axis=1)  # [OUT, B]
    return np.ascontiguousarray(y.T, dtype=np.float32)


if __name__ == "__main__":
    rng = np.random.default_rng(0)
    x = rng.standard_normal((B, IN), dtype=np.float32)
    W_in = (rng.standard_normal((H, IN), dtype=np.float32) / np.sqrt(IN))
    b_in_a = np.zeros(H, np.float32)
    Wd_a = rng.standard_normal((L, H, D, H), dtype=np.float32) * 0.1
    soma_a = rng.standard_normal((L, H, D), dtype=np.float32) * 0.1
    W_out = rng.standard_normal((OUT, H), dtype=np.float32) / np.sqrt(H)
    b_out_a = np.zeros(OUT, np.float32)
    y = kernel(x=x, W_in=W_in, b_in=b_in_a, Wd=Wd_a, soma=soma_a,
               W_out=W_out, b_out=b_out_a)
    print("kernel output:", y.shape, y.dtype, float(np.abs(y).max()))
